# revision 17
# baseline (speedup 1.0000x reference)
"""MinGRU (2-layer bidirectional) Bass kernel for Trainium2, 8 NeuronCores.

Strategy: data-parallel over batch (B=8 -> 1 batch element per core).
Per core the recurrence h_t = a_t*h_{t-1} + b_t (diagonal per channel) runs on
the Vector engine via tensor_tensor_scan with channels on partitions and time
on the free axis; matmuls run in float32r (full-rate fp32) on the PE; the
backward direction uses negative-stride access patterns so nothing is ever
physically reversed. Layer-0 forward states stay resident in SBUF; backward
states round-trip through DRAM. Layer-1 backward walks T descending right
behind layer-0 backward so the two pipelines overlap.
"""
import numpy as np
from contextlib import ExitStack

import concourse.bacc as bacc
import concourse.tile as tile
from concourse import mybir
from concourse import bass_utils

F32 = mybir.dt.float32
F32R = mybir.dt.float32r
SIG = mybir.ActivationFunctionType.Sigmoid
MULT = mybir.AluOpType.mult
ADD = mybir.AluOpType.add
MAX = mybir.AluOpType.max

T = 8192
D = 256
H = 256
BLK = 512
NBLK = T // BLK  # 16


def _build():
    nc = bacc.Bacc("TRN2", target_bir_lowering=False, debug=False)

    xs_d = nc.dram_tensor("xs", [T, D], F32, kind="ExternalInput").ap()
    w0f_d = nc.dram_tensor("w0f", [2 * H, D], F32, kind="ExternalInput").ap()
    w0b_d = nc.dram_tensor("w0b", [2 * H, D], F32, kind="ExternalInput").ap()
    w1f_d = nc.dram_tensor("w1f", [2 * H, 2 * H], F32, kind="ExternalInput").ap()
    w1b_d = nc.dram_tensor("w1b", [2 * H, 2 * H], F32, kind="ExternalInput").ap()
    ident_d = nc.dram_tensor("ident", [128, 128], F32, kind="ExternalInput").ap()

    out_d = nc.dram_tensor("out", [T, 2 * H], F32, kind="ExternalOutput").ap()
    hlast_d = nc.dram_tensor("hlast", [4, H], F32, kind="ExternalOutput").ap()

    with tile.TileContext(nc) as tc, ExitStack() as ctx:
        persist = ctx.enter_context(tc.tile_pool(name="persist", bufs=1))
        dram = ctx.enter_context(tc.tile_pool(name="dram", bufs=1, space="DRAM"))

        ident = persist.tile([128, 128], F32, tag="ident")
        nc.sync.dma_start(ident[:], ident_d[:])

        # ---- weights: load, PE-transpose, round to f32r ----
        def prep_w(w_d, K, name, wst, wps):
            kt = K // 128
            outs = [persist.tile([128, 512], F32R, tag=f"{name}_{kk}", name=f"{name}_{kk}")
                    for kk in range(kt)]
            stages = []
            for m in range(4):
                st = wst.tile([128, K], F32, tag=f"wst{m}")
                nc.sync.dma_start(st[:], w_d[m * 128:(m + 1) * 128, :])
                stages.append(st)
            for kk in range(kt):
                ps = wps.tile([128, 512], F32, tag="wps")
                for m in range(4):
                    nc.tensor.transpose(
                        ps[:, m * 128:(m + 1) * 128],
                        stages[m][:, kk * 128:(kk + 1) * 128],
                        ident[:],
                    )
                nc.scalar.copy(outs[kk][:], ps[:])
            return outs

        with tc.tile_pool(name="wst", bufs=2) as wst, \
             tc.tile_pool(name="wps", bufs=2, space="PSUM") as wps:
            w0fT = prep_w(w0f_d, D, "w0f", wst, wps)
            w0bT = prep_w(w0b_d, D, "w0b", wst, wps)
            w1fT = prep_w(w1f_d, 2 * H, "w1f", wst, wps)
            w1bT = prep_w(w1b_d, 2 * H, "w1b", wst, wps)

        # layer-0 forward hidden states stay resident (rhs of layer-1 matmuls)
        h0f = [persist.tile([128, T], F32R, tag=f"h0f{p}", name=f"h0f{p}") for p in range(2)]
        # resident transposed input [d, t] (f32r); allocated in the layer-0
        # scope below so its SBUF is reclaimed for layer 1
        xT = []
        # layer-0 backward states round-trip through DRAM
        h0b_scr = [dram.tile([128, T], F32R, tag=f"h0bscr{p}", name=f"h0bscr{p}")
                   for p in range(2)]

        ab = ctx.enter_context(tc.tile_pool(name="ab", bufs=2))
        sup = ctx.enter_context(tc.tile_pool(name="sup", bufs=2))

        # hg: 2-bank psum tile [h | gate]; one 1024-wide sigmoid -> [u | s],
        # a = 1-s (DVE), m = (h+0.5) max u in-place (DVE), b = s*m (GpSimd).
        def act_block(hg, half, a_on_pool=False):
            su = sup.tile([128, 2 * BLK], F32, tag=f"su{half}")
            a_t = ab.tile([128, BLK], F32, tag=f"a{half}")
            b_t = ab.tile([128, BLK], F32, tag=f"b{half}")
            nc.scalar.activation(su[:], hg[:], SIG)
            eng = nc.gpsimd if a_on_pool else nc.vector
            eng.tensor_scalar(a_t[:], su[:, BLK:], -1.0, 1.0, MULT, ADD)
            nc.vector.scalar_tensor_tensor(su[:, :BLK], hg[:, :BLK], 0.5, su[:, :BLK],
                                           ADD, MAX)
            nc.gpsimd.tensor_tensor(b_t[:], su[:, BLK:], su[:, :BLK], MULT)
            return a_t, b_t

        # stream x block k from DRAM and transpose into resident xT
        def load_xT(k, xstage, xtp):
            stg = xstage.tile([128, 4, D], F32, tag="xs")
            nc.sync.dma_start(
                stg[:], xs_d[k * BLK:(k + 1) * BLK, :].rearrange("(i p) d -> p i d", p=128))
            ps = xtp.tile([128, 2 * BLK], F32, tag="xtp")
            for p in range(2):
                for i in range(4):
                    nc.tensor.transpose(
                        ps[:, p * BLK + i * 128: p * BLK + (i + 1) * 128],
                        stg[:, i, p * 128:(p + 1) * 128],
                        ident[:],
                    )
            for p in range(2):
                nc.scalar.copy(xT[p][:, k * BLK:(k + 1) * BLK], ps[:, p * BLK:(p + 1) * BLK])

        def l0_mm(hg, wT, half, k):
            for off, mt in ((0, half), (BLK, half + 2)):
                for kk in range(2):
                    nc.tensor.matmul(hg[:, off:off + BLK],
                                     wT[kk][:, mt * 128:(mt + 1) * 128],
                                     xT[kk][:, k * BLK:(k + 1) * BLK],
                                     start=(kk == 0), stop=(kk == 1))

        def l1_mm(hg, wT, half, k, bwd_rhs):
            for off, mt in ((0, half), (BLK, half + 2)):
                for kk in range(4):
                    rhs = (h0f[kk][:, k * BLK:(k + 1) * BLK] if kk < 2
                           else bwd_rhs[kk - 2])
                    nc.tensor.matmul(hg[:, off:off + BLK],
                                     wT[kk][:, mt * 128:(mt + 1) * 128],
                                     rhs, start=(kk == 0), stop=(kk == 3))

        def store_block(h1s, k, hoff):
            for i in range(2):
                otp = otpp.tile([128, 2 * H], F32, tag="otp")
                for j in range(2):
                    for half in range(2):
                        nc.tensor.transpose(
                            otp[:, j * H + half * 128: j * H + (half + 1) * 128],
                            h1s[half][:, (2 * i + j) * 128:(2 * i + j + 1) * 128],
                            ident[:],
                        )
                ost = ostp.tile([128, 2 * H], F32, tag="ost")
                nc.scalar.copy(ost[:], otp[:])
                nc.sync.dma_start(
                    out_d[k * BLK + 2 * i * 128: k * BLK + (2 * i + 2) * 128,
                          hoff:hoff + H].rearrange("(j p) h -> p j h", p=128),
                    ost[:].rearrange("p (j h) -> p j h", j=2))

        def dma_hlast(row, half, src_col_ap):
            nc.sync.dma_start(
                hlast_d[row:row + 1, half * 128:(half + 1) * 128].rearrange("a b -> b a"),
                src_col_ap)

        # ---- phase 1: layer-0 forward (ascending); builds resident xT ----
        l0ctx = ExitStack()
        hgp = l0ctx.enter_context(tc.tile_pool(name="hgp", bufs=2, space="PSUM"))
        xtpool = l0ctx.enter_context(tc.tile_pool(name="xtpool", bufs=1))
        xT.extend(xtpool.tile([128, T], F32R, tag=f"xT{p}", name=f"xT{p}")
                  for p in range(2))
        xstage = l0ctx.enter_context(tc.tile_pool(name="xstage", bufs=2))
        xtp = l0ctx.enter_context(tc.tile_pool(name="xtp", bufs=1, space="PSUM"))
        hbp = l0ctx.enter_context(tc.tile_pool(name="hb", bufs=3))
        for k in range(NBLK):
            load_xT(k, xstage, xtp)
            for half in range(2):
                hg = hgp.tile([128, 2 * BLK], F32, tag="hg")
                l0_mm(hg, w0fT, half, k)
                a_t, b_t = act_block(hg, half, a_on_pool=True)
                init = 0.0 if k == 0 else h0f[half][:, k * BLK - 1: k * BLK]
                nc.vector.tensor_tensor_scan(
                    h0f[half][:, k * BLK:(k + 1) * BLK], a_t[:], b_t[:], init, MULT, ADD)
        for half in range(2):
            dma_hlast(0, half, h0f[half][:, T - 1: T].bitcast(F32))

        # ---- phase 2: layer-0 backward (descending, reversed-AP scans) ----
        hb_prev = [None, None]
        for k in range(NBLK - 1, -1, -1):
            for half in range(2):
                hg = hgp.tile([128, 2 * BLK], F32, tag="hg")
                l0_mm(hg, w0bT, half, k)
                a_t, b_t = act_block(hg, half, a_on_pool=True)
                hb = hbp.tile([128, BLK], F32R, tag=f"hb{half}")
                init = 0.0 if k == NBLK - 1 else hb_prev[half][:, 0:1]
                nc.vector.tensor_tensor_scan(
                    hb[:, ::-1], a_t[:, ::-1], b_t[:, ::-1], init, MULT, ADD)
                hb_prev[half] = hb
                nc.sync.dma_start(h0b_scr[half][:, k * BLK:(k + 1) * BLK], hb[:])
                if k == 0:
                    dma_hlast(1, half, hb[:, 0:1].bitcast(F32))

        l0ctx.close()

        # ---- phases 3+4: layer 1, backward then forward ----
        otpp = ctx.enter_context(tc.tile_pool(name="otp", bufs=2, space="PSUM"))
        hg1p = ctx.enter_context(tc.tile_pool(name="hg1p", bufs=3, space="PSUM"))
        h1p = ctx.enter_context(tc.tile_pool(name="h1", bufs=3))
        ostp = ctx.enter_context(tc.tile_pool(name="ost", bufs=2))
        rbp = ctx.enter_context(tc.tile_pool(name="rb", bufs=2))

        def l1_dir(wT, reverse):
            hoff = H if reverse else 0
            hl_row = 3 if reverse else 2
            h1_prev = [None, None]
            ks = range(NBLK - 1, -1, -1) if reverse else range(NBLK)
            for k in ks:
                rbs = []
                for p in range(2):
                    rb = rbp.tile([128, BLK], F32R, tag=f"rb{p}")
                    nc.sync.dma_start(rb[:], h0b_scr[p][:, k * BLK:(k + 1) * BLK])
                    rbs.append(rb[:])
                h1s = []
                for half in range(2):
                    hg = hg1p.tile([128, 2 * BLK], F32, tag="hg1")
                    l1_mm(hg, wT, half, k, rbs)
                    a_t, b_t = act_block(hg, half)
                    h1 = h1p.tile([128, BLK], F32, tag=f"h1_{half}")
                    if reverse:
                        init = 0.0 if k == NBLK - 1 else h1_prev[half][:, 0:1]
                        nc.vector.tensor_tensor_scan(
                            h1[:, ::-1], a_t[:, ::-1], b_t[:, ::-1], init, MULT, ADD)
                    else:
                        init = 0.0 if k == 0 else h1_prev[half][:, BLK - 1: BLK]
                        nc.vector.tensor_tensor_scan(h1[:], a_t[:], b_t[:], init, MULT, ADD)
                    h1_prev[half] = h1
                    h1s.append(h1)
                    if (not reverse and k == NBLK - 1) or (reverse and k == 0):
                        col = 0 if reverse else BLK - 1
                        dma_hlast(hl_row, half, h1[:, col:col + 1])
                store_block(h1s, k, hoff)

        l1_dir(w1bT, reverse=True)
        l1_dir(w1fT, reverse=False)

    nc.compile()
    return nc


_NC = None
last_results = None


def kernel(x, W0f, W0b, W1f, W1b):
    global _NC, last_results
    if _NC is None:
        _NC = _build()
    B = x.shape[0]
    ident = np.eye(128, dtype=np.float32)
    in_maps = []
    for b in range(B):
        in_maps.append(dict(
            xs=np.ascontiguousarray(x[b], np.float32),
            w0f=np.ascontiguousarray(W0f, np.float32),
            w0b=np.ascontiguousarray(W0b, np.float32),
            w1f=np.ascontiguousarray(W1f, np.float32),
            w1b=np.ascontiguousarray(W1b, np.float32),
            ident=ident,
        ))
    res = bass_utils.run_bass_kernel_spmd(_NC, in_maps, list(range(B)))
    last_results = res
    out = np.stack([res.results[b]["out"] for b in range(B)])        # [B, T, 2H]
    hl = np.stack([res.results[b]["hlast"] for b in range(B)], 1)    # [4, B, H]
    return out, hl


# revision 21
# speedup vs baseline: 1.1616x; 1.1616x over previous
"""MinGRU (2-layer bidirectional) Bass kernel for Trainium2, 8 NeuronCores.

Strategy: data-parallel over batch (B=8 -> 1 batch element per core).
Per core the recurrence h_t = a_t*h_{t-1} + b_t (diagonal per channel) runs on
the Vector engine via tensor_tensor_scan with channels on partitions and time
on the free axis; matmuls run in float32r (full-rate fp32) on the PE; the
backward direction uses negative-stride access patterns so nothing is ever
physically reversed. Layer-0 forward states stay resident in SBUF; backward
states round-trip through DRAM. Layer-1 backward walks T descending right
behind layer-0 backward so the two pipelines overlap.
"""
import numpy as np
from contextlib import ExitStack

import concourse.bacc as bacc
import concourse.tile as tile
from concourse import mybir
from concourse import bass_utils

F32 = mybir.dt.float32
F32R = mybir.dt.float32r
SIG = mybir.ActivationFunctionType.Sigmoid
MULT = mybir.AluOpType.mult
ADD = mybir.AluOpType.add
MAX = mybir.AluOpType.max

T = 8192
D = 256
H = 256
BLK = 512
NBLK = T // BLK  # 16


def _build():
    nc = bacc.Bacc("TRN2", target_bir_lowering=False, debug=False)

    xs_d = nc.dram_tensor("xs", [T, D], F32, kind="ExternalInput").ap()
    w0f_d = nc.dram_tensor("w0f", [2 * H, D], F32, kind="ExternalInput").ap()
    w0b_d = nc.dram_tensor("w0b", [2 * H, D], F32, kind="ExternalInput").ap()
    w1f_d = nc.dram_tensor("w1f", [2 * H, 2 * H], F32, kind="ExternalInput").ap()
    w1b_d = nc.dram_tensor("w1b", [2 * H, 2 * H], F32, kind="ExternalInput").ap()
    ident_d = nc.dram_tensor("ident", [128, 128], F32, kind="ExternalInput").ap()

    out_d = nc.dram_tensor("out", [T, 2 * H], F32, kind="ExternalOutput").ap()
    hlast_d = nc.dram_tensor("hlast", [4, H], F32, kind="ExternalOutput").ap()

    with tile.TileContext(nc) as tc, ExitStack() as ctx:
        persist = ctx.enter_context(tc.tile_pool(name="persist", bufs=1))
        dram = ctx.enter_context(tc.tile_pool(name="dram", bufs=1, space="DRAM"))

        ident = persist.tile([128, 128], F32, tag="ident")
        nc.sync.dma_start(ident[:], ident_d[:])

        # ---- weights: load, PE-transpose, round to f32r ----
        def prep_w(w_d, K, name, wst, wps):
            kt = K // 128
            outs = [persist.tile([128, 512], F32R, tag=f"{name}_{kk}", name=f"{name}_{kk}")
                    for kk in range(kt)]
            stages = []
            for m in range(4):
                st = wst.tile([128, K], F32, tag=f"wst{m}")
                nc.sync.dma_start(st[:], w_d[m * 128:(m + 1) * 128, :])
                stages.append(st)
            for kk in range(kt):
                ps = wps.tile([128, 512], F32, tag="wps")
                for m in range(4):
                    nc.tensor.transpose(
                        ps[:, m * 128:(m + 1) * 128],
                        stages[m][:, kk * 128:(kk + 1) * 128],
                        ident[:],
                    )
                nc.scalar.copy(outs[kk][:], ps[:])
            return outs

        with tc.tile_pool(name="wst", bufs=2) as wst, \
             tc.tile_pool(name="wps", bufs=2, space="PSUM") as wps:
            w0fT = prep_w(w0f_d, D, "w0f", wst, wps)
            w0bT = prep_w(w0b_d, D, "w0b", wst, wps)
            w1fT = prep_w(w1f_d, 2 * H, "w1f", wst, wps)
            w1bT = prep_w(w1b_d, 2 * H, "w1b", wst, wps)

        # layer-0 forward hidden states stay resident (rhs of layer-1 matmuls)
        h0f = [persist.tile([128, T], F32R, tag=f"h0f{p}", name=f"h0f{p}") for p in range(2)]
        # resident transposed input [d, t] (f32r); allocated in the layer-0
        # scope below so its SBUF is reclaimed for layer 1
        xT = []
        # layer-0 backward states round-trip through DRAM
        h0b_scr = [dram.tile([128, T], F32R, tag=f"h0bscr{p}", name=f"h0bscr{p}")
                   for p in range(2)]

        ab = ctx.enter_context(tc.tile_pool(name="ab", bufs=2))
        sup = ctx.enter_context(tc.tile_pool(name="sup", bufs=2))

        # hg: 2-bank psum tile [h | gate]; one 1024-wide sigmoid -> [u | s],
        # a = 1-s (DVE), m = (h+0.5) max u in-place (DVE), b = s*m (GpSimd).
        def act_block(hg, half, a_on_pool=False):
            su = sup.tile([128, 2 * BLK], F32, tag=f"su{half}")
            a_t = ab.tile([128, BLK], F32, tag=f"a{half}")
            b_t = ab.tile([128, BLK], F32, tag=f"b{half}")
            nc.scalar.activation(su[:], hg[:], SIG)
            eng = nc.gpsimd if a_on_pool else nc.vector
            eng.tensor_scalar(a_t[:], su[:, BLK:], -1.0, 1.0, MULT, ADD)
            nc.vector.scalar_tensor_tensor(su[:, :BLK], hg[:, :BLK], 0.5, su[:, :BLK],
                                           ADD, MAX)
            nc.gpsimd.tensor_tensor(b_t[:], su[:, BLK:], su[:, :BLK], MULT)
            return a_t, b_t

        # stream x block k from DRAM and transpose into resident xT
        def load_xT(k, xstage, xtp):
            stg = xstage.tile([128, 4, D], F32, tag="xs")
            nc.sync.dma_start(
                stg[:], xs_d[k * BLK:(k + 1) * BLK, :].rearrange("(i p) d -> p i d", p=128))
            ps = xtp.tile([128, 2 * BLK], F32, tag="xtp")
            for p in range(2):
                for i in range(4):
                    nc.tensor.transpose(
                        ps[:, p * BLK + i * 128: p * BLK + (i + 1) * 128],
                        stg[:, i, p * 128:(p + 1) * 128],
                        ident[:],
                    )
            for p in range(2):
                nc.scalar.copy(xT[p][:, k * BLK:(k + 1) * BLK], ps[:, p * BLK:(p + 1) * BLK])

        def l0_mm(hg, wT, half, k):
            for off, mt in ((0, half), (BLK, half + 2)):
                for kk in range(2):
                    nc.tensor.matmul(hg[:, off:off + BLK],
                                     wT[kk][:, mt * 128:(mt + 1) * 128],
                                     xT[kk][:, k * BLK:(k + 1) * BLK],
                                     start=(kk == 0), stop=(kk == 1))

        def l1_mm(hg, wT, half, k, bwd_rhs):
            for off, mt in ((0, half), (BLK, half + 2)):
                for kk in range(4):
                    rhs = (h0f[kk][:, k * BLK:(k + 1) * BLK] if kk < 2
                           else bwd_rhs[kk - 2])
                    nc.tensor.matmul(hg[:, off:off + BLK],
                                     wT[kk][:, mt * 128:(mt + 1) * 128],
                                     rhs, start=(kk == 0), stop=(kk == 3))

        def store_block(h1s, k, hoff):
            for i in range(2):
                otp = otpp.tile([128, 2 * H], F32, tag="otp")
                for j in range(2):
                    for half in range(2):
                        nc.tensor.transpose(
                            otp[:, j * H + half * 128: j * H + (half + 1) * 128],
                            h1s[half][:, (2 * i + j) * 128:(2 * i + j + 1) * 128],
                            ident[:],
                        )
                ost = ostp.tile([128, 2 * H], F32, tag="ost")
                nc.scalar.copy(ost[:], otp[:])
                nc.sync.dma_start(
                    out_d[k * BLK + 2 * i * 128: k * BLK + (2 * i + 2) * 128,
                          hoff:hoff + H].rearrange("(j p) h -> p j h", p=128),
                    ost[:].rearrange("p (j h) -> p j h", j=2))

        def dma_hlast(row, half, src_col_ap):
            nc.sync.dma_start(
                hlast_d[row:row + 1, half * 128:(half + 1) * 128].rearrange("a b -> b a"),
                src_col_ap)

        # ---- phase 1: layer-0 forward (ascending); builds resident xT ----
        l0ctx = ExitStack()
        hgp = l0ctx.enter_context(tc.tile_pool(name="hgp", bufs=2, space="PSUM"))
        xtpool = l0ctx.enter_context(tc.tile_pool(name="xtpool", bufs=1))
        xT.extend(xtpool.tile([128, T], F32R, tag=f"xT{p}", name=f"xT{p}")
                  for p in range(2))
        xstage = l0ctx.enter_context(tc.tile_pool(name="xstage", bufs=2))
        xtp = l0ctx.enter_context(tc.tile_pool(name="xtp", bufs=1, space="PSUM"))
        hbp = l0ctx.enter_context(tc.tile_pool(name="hb", bufs=3))
        for k in range(NBLK):
            load_xT(k, xstage, xtp)
            for half in range(2):
                hg = hgp.tile([128, 2 * BLK], F32, tag="hg")
                l0_mm(hg, w0fT, half, k)
                a_t, b_t = act_block(hg, half, a_on_pool=True)
                init = 0.0 if k == 0 else h0f[half][:, k * BLK - 1: k * BLK]
                nc.vector.tensor_tensor_scan(
                    h0f[half][:, k * BLK:(k + 1) * BLK], a_t[:], b_t[:], init, MULT, ADD)
        for half in range(2):
            dma_hlast(0, half, h0f[half][:, T - 1: T].bitcast(F32))

        # ---- phase 2: layer-0 backward (descending, reversed-AP scans) ----
        hb_prev = [None, None]
        for k in range(NBLK - 1, -1, -1):
            for half in range(2):
                hg = hgp.tile([128, 2 * BLK], F32, tag="hg")
                l0_mm(hg, w0bT, half, k)
                a_t, b_t = act_block(hg, half, a_on_pool=True)
                hb = hbp.tile([128, BLK], F32R, tag=f"hb{half}")
                init = 0.0 if k == NBLK - 1 else hb_prev[half][:, 0:1]
                nc.vector.tensor_tensor_scan(
                    hb[:, ::-1], a_t[:, ::-1], b_t[:, ::-1], init, MULT, ADD)
                hb_prev[half] = hb
                nc.sync.dma_start(h0b_scr[half][:, k * BLK:(k + 1) * BLK], hb[:])
                if k == 0:
                    dma_hlast(1, half, hb[:, 0:1].bitcast(F32))

        l0ctx.close()

        # ---- phases 3+4: layer 1, backward then forward ----
        otpp = ctx.enter_context(tc.tile_pool(name="otp", bufs=2, space="PSUM"))
        hg1p = ctx.enter_context(tc.tile_pool(name="hg1p", bufs=3, space="PSUM"))
        h1p = ctx.enter_context(tc.tile_pool(name="h1", bufs=4))
        ostp = ctx.enter_context(tc.tile_pool(name="ost", bufs=3))
        rbp = ctx.enter_context(tc.tile_pool(name="rb", bufs=3))

        def l1_dir(wT, reverse):
            hoff = H if reverse else 0
            hl_row = 3 if reverse else 2
            h1_prev = [None, None]
            ks = range(NBLK - 1, -1, -1) if reverse else range(NBLK)
            for k in ks:
                rbs = []
                for p in range(2):
                    rb = rbp.tile([128, BLK], F32R, tag=f"rb{p}")
                    nc.sync.dma_start(rb[:], h0b_scr[p][:, k * BLK:(k + 1) * BLK])
                    rbs.append(rb[:])
                h1s = []
                for half in range(2):
                    hg = hg1p.tile([128, 2 * BLK], F32, tag="hg1")
                    l1_mm(hg, wT, half, k, rbs)
                    a_t, b_t = act_block(hg, half)
                    h1 = h1p.tile([128, BLK], F32, tag=f"h1_{half}")
                    if reverse:
                        init = 0.0 if k == NBLK - 1 else h1_prev[half][:, 0:1]
                        nc.vector.tensor_tensor_scan(
                            h1[:, ::-1], a_t[:, ::-1], b_t[:, ::-1], init, MULT, ADD)
                    else:
                        init = 0.0 if k == 0 else h1_prev[half][:, BLK - 1: BLK]
                        nc.vector.tensor_tensor_scan(h1[:], a_t[:], b_t[:], init, MULT, ADD)
                    h1_prev[half] = h1
                    h1s.append(h1)
                    if (not reverse and k == NBLK - 1) or (reverse and k == 0):
                        col = 0 if reverse else BLK - 1
                        dma_hlast(hl_row, half, h1[:, col:col + 1])
                store_block(h1s, k, hoff)

        l1_dir(w1bT, reverse=True)
        l1_dir(w1fT, reverse=False)

    nc.compile()
    return nc


_NC = None
last_results = None


def kernel(x, W0f, W0b, W1f, W1b):
    global _NC, last_results
    if _NC is None:
        _NC = _build()
    B = x.shape[0]
    ident = np.eye(128, dtype=np.float32)
    in_maps = []
    for b in range(B):
        in_maps.append(dict(
            xs=np.ascontiguousarray(x[b], np.float32),
            w0f=np.ascontiguousarray(W0f, np.float32),
            w0b=np.ascontiguousarray(W0b, np.float32),
            w1f=np.ascontiguousarray(W1f, np.float32),
            w1b=np.ascontiguousarray(W1b, np.float32),
            ident=ident,
        ))
    res = bass_utils.run_bass_kernel_spmd(_NC, in_maps, list(range(B)))
    last_results = res
    out = np.stack([res.results[b]["out"] for b in range(B)])        # [B, T, 2H]
    hl = np.stack([res.results[b]["hlast"] for b in range(B)], 1)    # [4, B, H]
    return out, hl


# revision 25
# speedup vs baseline: 1.2118x; 1.0433x over previous
"""MinGRU (2-layer bidirectional) Bass kernel for Trainium2, 8 NeuronCores.

Strategy: data-parallel over batch (B=8 -> 1 batch element per core).
Per core the recurrence h_t = a_t*h_{t-1} + b_t (diagonal per channel) runs on
the Vector engine via tensor_tensor_scan with channels on partitions and time
on the free axis; matmuls run in float32r (full-rate fp32) on the PE; the
backward direction uses negative-stride access patterns so nothing is ever
physically reversed. Layer-0 forward states stay resident in SBUF; backward
states round-trip through DRAM. Layer-1 backward walks T descending right
behind layer-0 backward so the two pipelines overlap.
"""
import numpy as np
from contextlib import ExitStack

import concourse.bacc as bacc
import concourse.tile as tile
from concourse import mybir
from concourse import bass_utils

F32 = mybir.dt.float32
F32R = mybir.dt.float32r
SIG = mybir.ActivationFunctionType.Sigmoid
MULT = mybir.AluOpType.mult
ADD = mybir.AluOpType.add
MAX = mybir.AluOpType.max

T = 8192
D = 256
H = 256
BLK = 512
NBLK = T // BLK  # 16


def _build():
    nc = bacc.Bacc("TRN2", target_bir_lowering=False, debug=False)

    xs_d = nc.dram_tensor("xs", [T, D], F32, kind="ExternalInput").ap()
    w0f_d = nc.dram_tensor("w0f", [2 * H, D], F32, kind="ExternalInput").ap()
    w0b_d = nc.dram_tensor("w0b", [2 * H, D], F32, kind="ExternalInput").ap()
    w1f_d = nc.dram_tensor("w1f", [2 * H, 2 * H], F32, kind="ExternalInput").ap()
    w1b_d = nc.dram_tensor("w1b", [2 * H, 2 * H], F32, kind="ExternalInput").ap()
    ident_d = nc.dram_tensor("ident", [128, 128], F32, kind="ExternalInput").ap()

    out_d = nc.dram_tensor("out", [T, 2 * H], F32, kind="ExternalOutput").ap()
    hlast_d = nc.dram_tensor("hlast", [4, H], F32, kind="ExternalOutput").ap()

    with tile.TileContext(nc) as tc, ExitStack() as ctx:
        persist = ctx.enter_context(tc.tile_pool(name="persist", bufs=1))
        dram = ctx.enter_context(tc.tile_pool(name="dram", bufs=1, space="DRAM"))

        ident = persist.tile([128, 128], F32, tag="ident")
        nc.sync.dma_start(ident[:], ident_d[:])

        # ---- weights: load, PE-transpose, round to f32r ----
        def prep_w(w_d, K, name, wst, wps):
            kt = K // 128
            outs = [persist.tile([128, 512], F32R, tag=f"{name}_{kk}", name=f"{name}_{kk}")
                    for kk in range(kt)]
            stages = []
            for m in range(4):
                st = wst.tile([128, K], F32, tag=f"wst{m}")
                nc.sync.dma_start(st[:], w_d[m * 128:(m + 1) * 128, :])
                stages.append(st)
            for kk in range(kt):
                ps = wps.tile([128, 512], F32, tag="wps")
                for m in range(4):
                    nc.tensor.transpose(
                        ps[:, m * 128:(m + 1) * 128],
                        stages[m][:, kk * 128:(kk + 1) * 128],
                        ident[:],
                    )
                nc.scalar.copy(outs[kk][:], ps[:])
            return outs

        with tc.tile_pool(name="wst", bufs=2) as wst, \
             tc.tile_pool(name="wps", bufs=2, space="PSUM") as wps:
            w0fT = prep_w(w0f_d, D, "w0f", wst, wps)
            w0bT = prep_w(w0b_d, D, "w0b", wst, wps)
            w1fT = prep_w(w1f_d, 2 * H, "w1f", wst, wps)
            w1bT = prep_w(w1b_d, 2 * H, "w1b", wst, wps)

        # layer-0 forward hidden states stay resident (rhs of layer-1 matmuls)
        h0f = [persist.tile([128, T], F32R, tag=f"h0f{p}", name=f"h0f{p}") for p in range(2)]
        # layer-0 backward states round-trip through DRAM
        h0b_scr = [dram.tile([128, T], F32R, tag=f"h0bscr{p}", name=f"h0bscr{p}")
                   for p in range(2)]

        ab = ctx.enter_context(tc.tile_pool(name="ab", bufs=3))
        sup = ctx.enter_context(tc.tile_pool(name="sup", bufs=3))

        # hg: 2-bank psum tile [h | gate]; one 1024-wide sigmoid -> [u | s],
        # a = 1-s (DVE), m = (h+0.5) max u in-place (DVE), b = s*m (GpSimd).
        def act_block(hg, half, a_on_pool=False, dir_tag=""):
            su = sup.tile([128, 2 * BLK], F32, tag=f"su{half}", name="su")
            a_t = ab.tile([128, BLK], F32, tag=f"a{half}", name="a_t")
            b_t = ab.tile([128, BLK], F32, tag=f"b{half}", name="b_t")
            nc.scalar.activation(su[:], hg[:], SIG)
            eng = nc.gpsimd if a_on_pool else nc.vector
            eng.tensor_scalar(a_t[:], su[:, BLK:], -1.0, 1.0, MULT, ADD)
            nc.vector.scalar_tensor_tensor(su[:, :BLK], hg[:, :BLK], 0.5, su[:, :BLK],
                                           ADD, MAX)
            nc.gpsimd.tensor_tensor(b_t[:], su[:, BLK:], su[:, :BLK], MULT)
            return a_t, b_t

        # stream x block k from DRAM, transpose to a [d, t] f32r tile
        def load_xT(k, xstage, xtp, xtsb, dir_tag):
            stg = xstage.tile([128, 4, D], F32, tag=f"xs{dir_tag}", name="stg")
            nc.sync.dma_start(
                stg[:], xs_d[k * BLK:(k + 1) * BLK, :].rearrange("(i p) d -> p i d", p=128))
            ps = xtp.tile([128, 2 * BLK], F32, tag="xtp", name="ps")
            for p in range(2):
                for i in range(4):
                    nc.tensor.transpose(
                        ps[:, p * BLK + i * 128: p * BLK + (i + 1) * 128],
                        stg[:, i, p * 128:(p + 1) * 128],
                        ident[:],
                    )
            xt = xtsb.tile([128, 2 * BLK], F32R, tag=f"xt{dir_tag}", name="xt")
            nc.scalar.copy(xt[:], ps[:])
            return xt

        def l0_mm(hg, wT, half, xt):
            for off, mt in ((0, half), (BLK, half + 2)):
                for kk in range(2):
                    nc.tensor.matmul(hg[:, off:off + BLK],
                                     wT[kk][:, mt * 128:(mt + 1) * 128],
                                     xt[:, kk * BLK:(kk + 1) * BLK],
                                     start=(kk == 0), stop=(kk == 1))

        def l1_mm(hg, wT, half, k, bwd_rhs):
            for off, mt in ((0, half), (BLK, half + 2)):
                for kk in range(4):
                    rhs = (h0f[kk][:, k * BLK:(k + 1) * BLK] if kk < 2
                           else bwd_rhs[kk - 2])
                    nc.tensor.matmul(hg[:, off:off + BLK],
                                     wT[kk][:, mt * 128:(mt + 1) * 128],
                                     rhs, start=(kk == 0), stop=(kk == 3))

        def store_block(h1s, k, hoff):
            for i in range(2):
                otp = otpp.tile([128, 2 * H], F32, tag="otp")
                for j in range(2):
                    for half in range(2):
                        nc.tensor.transpose(
                            otp[:, j * H + half * 128: j * H + (half + 1) * 128],
                            h1s[half][:, (2 * i + j) * 128:(2 * i + j + 1) * 128],
                            ident[:],
                        )
                ost = ostp.tile([128, 2 * H], F32, tag="ost")
                nc.scalar.copy(ost[:], otp[:])
                nc.sync.dma_start(
                    out_d[k * BLK + 2 * i * 128: k * BLK + (2 * i + 2) * 128,
                          hoff:hoff + H].rearrange("(j p) h -> p j h", p=128),
                    ost[:].rearrange("p (j h) -> p j h", j=2))

        def dma_hlast(row, half, src_col_ap):
            nc.sync.dma_start(
                hlast_d[row:row + 1, half * 128:(half + 1) * 128].rearrange("a b -> b a"),
                src_col_ap)

        # ---- layer 0: forward and backward run CONCURRENTLY ----
        # each direction streams its own copy of x; fwd ascends, bwd descends,
        # so neither waits on the other and the DVE interleaves both scan chains
        l0ctx = ExitStack()
        hgp = l0ctx.enter_context(tc.tile_pool(name="hgp", bufs=2, space="PSUM"))
        xtp = l0ctx.enter_context(tc.tile_pool(name="xtp", bufs=2, space="PSUM"))
        xstage = l0ctx.enter_context(tc.tile_pool(name="xstage", bufs=2))
        xtsb = l0ctx.enter_context(tc.tile_pool(name="xtsb", bufs=2))
        hbp = l0ctx.enter_context(tc.tile_pool(name="hb", bufs=3))
        hb_prev = [None, None]
        for j2 in range(NBLK):
            kf = j2
            kb = NBLK - 1 - j2
            # forward sub-step
            xt = load_xT(kf, xstage, xtp, xtsb, "f")
            for half in range(2):
                hg = hgp.tile([128, 2 * BLK], F32, tag="hg", name="hg")
                l0_mm(hg, w0fT, half, xt)
                a_t, b_t = act_block(hg, half, a_on_pool=True, dir_tag="f")
                init = 0.0 if kf == 0 else h0f[half][:, kf * BLK - 1: kf * BLK]
                nc.vector.tensor_tensor_scan(
                    h0f[half][:, kf * BLK:(kf + 1) * BLK], a_t[:], b_t[:], init, MULT, ADD)
            # backward sub-step
            xt = load_xT(kb, xstage, xtp, xtsb, "b")
            for half in range(2):
                hg = hgp.tile([128, 2 * BLK], F32, tag="hg", name="hg")
                l0_mm(hg, w0bT, half, xt)
                a_t, b_t = act_block(hg, half, a_on_pool=True, dir_tag="b")
                hb = hbp.tile([128, BLK], F32R, tag=f"hb{half}", name="hb")
                init = 0.0 if kb == NBLK - 1 else hb_prev[half][:, 0:1]
                nc.vector.tensor_tensor_scan(
                    hb[:, ::-1], a_t[:, ::-1], b_t[:, ::-1], init, MULT, ADD)
                hb_prev[half] = hb
                nc.sync.dma_start(h0b_scr[half][:, kb * BLK:(kb + 1) * BLK], hb[:])
                if kb == 0:
                    dma_hlast(1, half, hb[:, 0:1].bitcast(F32))
        for half in range(2):
            dma_hlast(0, half, h0f[half][:, T - 1: T].bitcast(F32))
        l0ctx.close()

        # ---- phases 3+4: layer 1, backward then forward ----
        otpp = ctx.enter_context(tc.tile_pool(name="otp", bufs=2, space="PSUM"))
        hg1p = ctx.enter_context(tc.tile_pool(name="hg1p", bufs=3, space="PSUM"))
        h1p = ctx.enter_context(tc.tile_pool(name="h1", bufs=4))
        ostp = ctx.enter_context(tc.tile_pool(name="ost", bufs=3))
        rbp = ctx.enter_context(tc.tile_pool(name="rb", bufs=3))

        def l1_dir(wT, reverse):
            hoff = H if reverse else 0
            hl_row = 3 if reverse else 2
            h1_prev = [None, None]
            ks = range(NBLK - 1, -1, -1) if reverse else range(NBLK)
            for k in ks:
                rbs = []
                for p in range(2):
                    rb = rbp.tile([128, BLK], F32R, tag=f"rb{p}")
                    nc.sync.dma_start(rb[:], h0b_scr[p][:, k * BLK:(k + 1) * BLK])
                    rbs.append(rb[:])
                h1s = []
                for half in range(2):
                    hg = hg1p.tile([128, 2 * BLK], F32, tag="hg1")
                    l1_mm(hg, wT, half, k, rbs)
                    a_t, b_t = act_block(hg, half)
                    h1 = h1p.tile([128, BLK], F32, tag=f"h1_{half}")
                    if reverse:
                        init = 0.0 if k == NBLK - 1 else h1_prev[half][:, 0:1]
                        nc.vector.tensor_tensor_scan(
                            h1[:, ::-1], a_t[:, ::-1], b_t[:, ::-1], init, MULT, ADD)
                    else:
                        init = 0.0 if k == 0 else h1_prev[half][:, BLK - 1: BLK]
                        nc.vector.tensor_tensor_scan(h1[:], a_t[:], b_t[:], init, MULT, ADD)
                    h1_prev[half] = h1
                    h1s.append(h1)
                    if (not reverse and k == NBLK - 1) or (reverse and k == 0):
                        col = 0 if reverse else BLK - 1
                        dma_hlast(hl_row, half, h1[:, col:col + 1])
                store_block(h1s, k, hoff)

        l1_dir(w1bT, reverse=True)
        l1_dir(w1fT, reverse=False)

    nc.compile()
    return nc


_NC = None
last_results = None


def kernel(x, W0f, W0b, W1f, W1b):
    global _NC, last_results
    if _NC is None:
        _NC = _build()
    B = x.shape[0]
    ident = np.eye(128, dtype=np.float32)
    in_maps = []
    for b in range(B):
        in_maps.append(dict(
            xs=np.ascontiguousarray(x[b], np.float32),
            w0f=np.ascontiguousarray(W0f, np.float32),
            w0b=np.ascontiguousarray(W0b, np.float32),
            w1f=np.ascontiguousarray(W1f, np.float32),
            w1b=np.ascontiguousarray(W1b, np.float32),
            ident=ident,
        ))
    res = bass_utils.run_bass_kernel_spmd(_NC, in_maps, list(range(B)))
    last_results = res
    out = np.stack([res.results[b]["out"] for b in range(B)])        # [B, T, 2H]
    hl = np.stack([res.results[b]["hlast"] for b in range(B)], 1)    # [4, B, H]
    return out, hl
